# revision 4
# baseline (speedup 1.0000x reference)
"""Trainium2 Bass kernel for nn_ExtractorMLP: per-edge MLP over gathered node
embeddings, data-parallel over edges across 8 NeuronCores (emb table + weights
replicated per core).

Per edge e: out = relu(relu(concat(emb[col[e]], emb[row[e]]) @ W1 + b1) @ W2 + b2) @ W3 + b3

v3 design: both endpoint gathers use the bulk SWDGE `dma_gather` custom DMA
(transpose=True), which amortizes the ~1us per-call SWDGE fixed overhead over
2048 indices per call and delivers tiles directly in [feature, edge] layout
(no PE transposes, no one-hot selection matmuls). dma_gather indices are int16,
so the node table (bf16) is split at row 25088 into lo/hi halves and each
core's edges are bucketed by (row_half, col_half) into 4 padded buckets; local
indices then fit in [0, 25088). The MLP runs in bf16 (weights stationary,
activations [feature, edge]); the concat is realized by PSUM accumulation of
the col/row partial products. Bias+ReLU passes alternate between ScalarE and
VectorE (fused add+max tensor_scalar) so neither engine gates the PE. Layer-3
[2, 512] results are staged per-chunk and b3 is added on the host during the
unpermute (b3 is spec'd zeros; kept general)."""

import sys

import numpy as np

N_NODES = 50000
HIDDEN = 128
N_EDGES = 800000
N_CORES = 8
E_SHARD = N_EDGES // N_CORES

SPLIT = 25088              # lo/hi table boundary (196*128)
TAB_PAD = 2 * SPLIT        # padded table rows (50176)
# dma_gather emits num_idxs/16 + 2 descriptors per SDMA engine and the
# per-(queue, engine) ring holds 64: num_idxs <= 896 (and % 128 == 0).
# HW-verified: 896 OK, 1024 faults the device.
G_CHUNK = 896              # edges per dma_gather call
BLOCK = 448                # edges per matmul block
CAP = 26880                # bucket capacity (30 chunks of 896)
N_BUCKETS = 4
CHUNKS_PER_BUCKET = CAP // G_CHUNK
N_CHUNKS = N_BUCKETS * CHUNKS_PER_BUCKET  # 120
E_PAD = N_BUCKETS * CAP    # 107520
BLK_PER_CHUNK = G_CHUNK // BLOCK  # 2

_REPO = "/opt/trn_rl_repo"
_prog_cache = {}
RUN_KWARGS = {}
LAST_RESULTS = None


def _build_program_v3(debug=False):
    if _REPO not in sys.path:
        sys.path.insert(0, _REPO)
    from concourse import bacc, mybir
    import concourse.tile as tile

    f32 = mybir.dt.float32
    bf16 = mybir.dt.bfloat16
    i16 = mybir.dt.int16
    Relu = mybir.ActivationFunctionType.Relu
    Ident = mybir.ActivationFunctionType.Identity
    ADD = mybir.AluOpType.add
    MAX = mybir.AluOpType.max

    nc = bacc.Bacc("TRN2", target_bir_lowering=False, debug=debug)
    emb16 = nc.dram_tensor("emb16", [TAB_PAD, HIDDEN], bf16, kind="ExternalInput")
    idx_col = nc.dram_tensor("idx_col", [N_CHUNKS, 128, G_CHUNK // 16], i16,
                             kind="ExternalInput")
    idx_row = nc.dram_tensor("idx_row", [N_CHUNKS, 128, G_CHUNK // 16], i16,
                             kind="ExternalInput")
    w1a = nc.dram_tensor("w1a", [HIDDEN, 4 * HIDDEN], bf16, kind="ExternalInput")
    w1b = nc.dram_tensor("w1b", [HIDDEN, 4 * HIDDEN], bf16, kind="ExternalInput")
    w2s = nc.dram_tensor("w2s", [HIDDEN, 4 * HIDDEN], bf16, kind="ExternalInput")
    w3t = nc.dram_tensor("w3t", [HIDDEN, 2], bf16, kind="ExternalInput")
    b1t = nc.dram_tensor("b1t", [128, 4], f32, kind="ExternalInput")
    b2t = nc.dram_tensor("b2t", [128, 1], f32, kind="ExternalInput")
    out_t = nc.dram_tensor("out_t", [2, E_PAD], f32, kind="ExternalOutput")

    with tile.TileContext(nc) as tc:
        with (
            tc.tile_pool(name="const", bufs=1) as cp,
            tc.tile_pool(name="idx", bufs=3) as ip,
            tc.tile_pool(name="gath", bufs=3) as gp,
            tc.tile_pool(name="act", bufs=3) as ap_,
            tc.tile_pool(name="outp", bufs=2) as op_,
            tc.tile_pool(name="ps_h1", bufs=2, space="PSUM") as psh1,
            tc.tile_pool(name="ps_h2", bufs=2, space="PSUM") as psh2,
            tc.tile_pool(name="ps_o", bufs=2, space="PSUM") as pso,
        ):
            w1a_sb = cp.tile([128, 512], bf16)
            nc.sync.dma_start(out=w1a_sb[:], in_=w1a[:])
            w1b_sb = cp.tile([128, 512], bf16)
            nc.sync.dma_start(out=w1b_sb[:], in_=w1b[:])
            w2_sb = cp.tile([128, 512], bf16)
            nc.sync.dma_start(out=w2_sb[:], in_=w2s[:])
            w3_sb = cp.tile([128, 2], bf16)
            nc.sync.dma_start(out=w3_sb[:], in_=w3t[:])
            b1_sb = cp.tile([128, 4], f32)
            nc.sync.dma_start(out=b1_sb[:], in_=b1t[:])
            b2_sb = cp.tile([128, 1], f32)
            nc.sync.dma_start(out=b2_sb[:], in_=b2t[:])

            for c in range(N_CHUNKS):
                bkt = c // CHUNKS_PER_BUCKET
                ch, rh = bkt & 1, bkt >> 1
                csrc = emb16[SPLIT:TAB_PAD] if ch else emb16[0:SPLIT]
                rsrc = emb16[SPLIT:TAB_PAD] if rh else emb16[0:SPLIT]

                ic = ip.tile([128, G_CHUNK // 16], i16, tag="ic")
                nc.sync.dma_start(out=ic[:], in_=idx_col[c])
                ir = ip.tile([128, G_CHUNK // 16], i16, tag="ir")
                nc.sync.dma_start(out=ir[:], in_=idx_row[c])

                colT = gp.tile([128, 1, G_CHUNK], bf16, tag="colT")
                nc.gpsimd.dma_gather(
                    colT[:], csrc, ic[:], G_CHUNK, G_CHUNK, HIDDEN,
                    transpose=True,
                )
                rowT = gp.tile([128, 1, G_CHUNK], bf16, tag="rowT")
                nc.gpsimd.dma_gather(
                    rowT[:], rsrc, ir[:], G_CHUNK, G_CHUNK, HIDDEN,
                    transpose=True,
                )

                o_sb = op_.tile([2, G_CHUNK], f32, tag="o_sb")
                for blk in range(BLK_PER_CHUNK):
                    sl = slice(blk * BLOCK, (blk + 1) * BLOCK)
                    h1T = ap_.tile([128, 4 * BLOCK], bf16, tag="h1T")
                    for m in range(4):
                        h1p = psh1.tile([128, BLOCK], f32, tag="h1p")
                        nc.tensor.matmul(
                            out=h1p[:],
                            lhsT=w1a_sb[:, m * 128:(m + 1) * 128],
                            rhs=colT[:, 0, sl],
                            start=True,
                            stop=False,
                        )
                        nc.tensor.matmul(
                            out=h1p[:],
                            lhsT=w1b_sb[:, m * 128:(m + 1) * 128],
                            rhs=rowT[:, 0, sl],
                            start=False,
                            stop=True,
                        )
                        dst = h1T[:, m * BLOCK:(m + 1) * BLOCK]
                        if m % 2 == 0:
                            nc.scalar.activation(
                                out=dst, in_=h1p[:], func=Relu,
                                bias=b1_sb[:, m:m + 1],
                            )
                        else:
                            nc.vector.tensor_scalar(
                                out=dst, in0=h1p[:],
                                scalar1=b1_sb[:, m:m + 1], scalar2=0.0,
                                op0=ADD, op1=MAX,
                            )

                    h2p = psh2.tile([128, BLOCK], f32, tag="h2p")
                    for k in range(4):
                        nc.tensor.matmul(
                            out=h2p[:],
                            lhsT=w2_sb[:, k * 128:(k + 1) * 128],
                            rhs=h1T[:, k * BLOCK:(k + 1) * BLOCK],
                            start=(k == 0),
                            stop=(k == 3),
                        )
                    h2T = ap_.tile([128, BLOCK], bf16, tag="h2T")
                    nc.vector.tensor_scalar(
                        out=h2T[:], in0=h2p[:],
                        scalar1=b2_sb[:, 0:1], scalar2=0.0,
                        op0=ADD, op1=MAX,
                    )

                    op = pso.tile([2, BLOCK], f32, tag="op")
                    nc.tensor.matmul(
                        out=op[:], lhsT=w3_sb[:], rhs=h2T[:],
                        start=True, stop=True,
                    )
                    nc.scalar.activation(out=o_sb[:, sl], in_=op[:], func=Ident)

                nc.sync.dma_start(
                    out=out_t[:, c * G_CHUNK:(c + 1) * G_CHUNK], in_=o_sb[:]
                )

    nc.compile()
    return nc


def _get_program():
    if "v3" not in _prog_cache:
        _prog_cache["v3"] = _build_program_v3()
    return _prog_cache["v3"]


def _gather_idx_layout(lidx):
    """[E_PAD] local indices -> [N_CHUNKS, 128, G_CHUNK//16] int16 in the
    dma_gather wrapped layout (element i of a chunk at [i%16, i//16],
    replicated across the 8 16-partition groups)."""
    a = lidx.astype(np.int16).reshape(N_CHUNKS, G_CHUNK // 16, 16)
    a = a.transpose(0, 2, 1)  # [N_CHUNKS, 16, G_CHUNK//16]
    return np.ascontiguousarray(np.tile(a, (1, 8, 1)))


def _marshal_core(col, row):
    """col/row: int32 [E_SHARD]. Returns (idx_col, idx_row, order, slots)."""
    bucket = (row >= SPLIT).astype(np.int64) * 2 + (col >= SPLIT)
    counts = np.bincount(bucket, minlength=N_BUCKETS)
    if counts.max() > CAP:
        raise RuntimeError(f"bucket overflow: {counts} > {CAP}")
    order = np.argsort(bucket, kind="stable")
    starts = np.concatenate([[0], np.cumsum(counts)[:-1]])
    within = np.arange(len(order)) - np.repeat(starts, counts)
    slots = np.repeat(np.arange(N_BUCKETS) * CAP, counts) + within

    lcol = np.zeros(E_PAD, np.int32)
    lrow = np.zeros(E_PAD, np.int32)
    bs = bucket[order]
    lcol[slots] = col[order] - SPLIT * (bs & 1)
    lrow[slots] = row[order] - SPLIT * (bs >> 1)
    return {
        "idx_col": _gather_idx_layout(lcol),
        "idx_row": _gather_idx_layout(lrow),
    }, order, slots


def kernel(emb, edge_index, W1, b1, W2, b2, W3, b3):
    if _REPO not in sys.path:
        sys.path.insert(0, _REPO)
    import ml_dtypes
    from concourse.bass_utils import run_bass_kernel_spmd

    bf = ml_dtypes.bfloat16
    emb = np.ascontiguousarray(np.asarray(emb, dtype=np.float32))
    emb16 = np.zeros((TAB_PAD, HIDDEN), bf)
    emb16[:N_NODES] = emb.astype(bf)
    ei = np.asarray(edge_index)
    col = ei[0].astype(np.int32)
    row = ei[1].astype(np.int32)
    W1 = np.asarray(W1, np.float32)
    w1a = np.ascontiguousarray(W1[:128].astype(bf))
    w1b = np.ascontiguousarray(W1[128:].astype(bf))
    W2 = np.asarray(W2, np.float32)
    # w2s[:, k*128:(k+1)*128] = W2[k*128:(k+1)*128, :]
    w2s = np.concatenate([W2[k * 128:(k + 1) * 128] for k in range(4)], axis=1)
    w2s = np.ascontiguousarray(w2s.astype(bf))
    w3t = np.ascontiguousarray(np.asarray(W3, np.float32).astype(bf))
    b1t = np.ascontiguousarray(np.asarray(b1, np.float32).reshape(4, 128).T)
    b2t = np.ascontiguousarray(np.asarray(b2, np.float32).reshape(128, 1))
    b3v = np.asarray(b3, np.float32).reshape(1, 2)

    in_maps = []
    unperm = []
    for i in range(N_CORES):
        m, order, slots = _marshal_core(
            col[i * E_SHARD:(i + 1) * E_SHARD],
            row[i * E_SHARD:(i + 1) * E_SHARD],
        )
        m.update(emb16=emb16, w1a=w1a, w1b=w1b, w2s=w2s, w3t=w3t,
                 b1t=b1t, b2t=b2t)
        in_maps.append(m)
        unperm.append((order, slots))

    nc = _get_program()
    try:
        res = run_bass_kernel_spmd(nc, in_maps, list(range(N_CORES)), **RUN_KWARGS)
    except Exception:
        import ctypes

        lib = ctypes.CDLL("/opt/axon/libaxon_pjrt.so")
        lib.axon_reset.restype = ctypes.c_int64
        lib.axon_reset()
        res = run_bass_kernel_spmd(nc, in_maps, list(range(N_CORES)), **RUN_KWARGS)
    global LAST_RESULTS
    LAST_RESULTS = res

    out = np.empty((N_EDGES, 2), np.float32)
    for i in range(N_CORES):
        ot = np.asarray(res.results[i]["out_t"], np.float32)  # [2, E_PAD]
        order, slots = unperm[i]
        shard = out[i * E_SHARD:(i + 1) * E_SHARD]
        shard[order] = ot.T[slots]
    out += b3v
    return out


# revision 6
# speedup vs baseline: 1.5347x; 1.5347x over previous
"""Trainium2 Bass kernel for nn_ExtractorMLP: per-edge MLP over gathered node
embeddings, data-parallel over edges across 8 NeuronCores (emb table + weights
replicated per core).

Per edge e: out = relu(relu(concat(emb[col[e]], emb[row[e]]) @ W1 + b1) @ W2 + b2) @ W3 + b3

v4 design: both endpoint gathers use the bulk SWDGE `dma_gather` custom DMA in
NON-transpose mode (contiguous 256B-row writes; the transpose mode's 2-byte
scattered SBUF writes measured ~30GB/s and made the dynamic-DMA queue the
bottleneck at 1.95ms). Gathered [128, 7, 128] tiles are transposed to
[feature, edge] on the PE (identity matmul, 4+3 groups batched per PSUM bank)
with paired PSUM->SBUF copies split across ScalarE/VectorE. dma_gather emits
num_idxs/16+2 descriptors per SDMA engine against a 64-deep ring, so chunks
are 896 edges (HW-verified: 1024 faults the device); col/row gathers ride
separate SWDGE queues. dma_gather indices are int16, so the bf16 node table is
split at row 25088 into lo/hi halves and each core's edges are bucketed by
(row_half, col_half) into 4 padded buckets; local indices then fit in
[0, 25088). The MLP runs in bf16 (weights stationary, activations
[feature, edge]); the concat is realized by PSUM accumulation of the col/row
partial products. Bias+ReLU passes alternate between ScalarE and VectorE
(fused add+max tensor_scalar). The chunk loop is software-pipelined one deep
(gather/transpose chunk c while the MLP consumes chunk c-1) so the PE never
waits on the copy latency. Layer-3 [2, 448] results are staged per-chunk and
b3 is added on the host during the unpermute."""

import sys

import numpy as np

N_NODES = 50000
HIDDEN = 128
N_EDGES = 800000
N_CORES = 8
E_SHARD = N_EDGES // N_CORES

SPLIT = 25088              # lo/hi table boundary (196*128)
TAB_PAD = 2 * SPLIT        # padded table rows (50176)
G_CHUNK = 896              # edges per dma_gather call (ring limit: <= 896)
NGRP = G_CHUNK // 128      # 7 transpose groups per gather
BLOCK = 448                # edges per matmul block
CAP = 26880                # bucket capacity (30 chunks of 896)
N_BUCKETS = 4
CHUNKS_PER_BUCKET = CAP // G_CHUNK
N_CHUNKS = N_BUCKETS * CHUNKS_PER_BUCKET  # 120
E_PAD = N_BUCKETS * CAP    # 107520
BLK_PER_CHUNK = G_CHUNK // BLOCK  # 2

_REPO = "/opt/trn_rl_repo"
_prog_cache = {}
RUN_KWARGS = {}
LAST_RESULTS = None


def _build_program_v4(debug=False):
    if _REPO not in sys.path:
        sys.path.insert(0, _REPO)
    from concourse import bacc, mybir
    import concourse.tile as tile
    from concourse.masks import make_identity

    f32 = mybir.dt.float32
    bf16 = mybir.dt.bfloat16
    i16 = mybir.dt.int16
    Relu = mybir.ActivationFunctionType.Relu
    Ident = mybir.ActivationFunctionType.Identity
    ADD = mybir.AluOpType.add
    MAX = mybir.AluOpType.max

    nc = bacc.Bacc("TRN2", target_bir_lowering=False, debug=debug,
                   num_swdge_queues=2)
    emb16 = nc.dram_tensor("emb16", [TAB_PAD, HIDDEN], bf16, kind="ExternalInput")
    idx_col = nc.dram_tensor("idx_col", [N_CHUNKS, 128, G_CHUNK // 16], i16,
                             kind="ExternalInput")
    idx_row = nc.dram_tensor("idx_row", [N_CHUNKS, 128, G_CHUNK // 16], i16,
                             kind="ExternalInput")
    w1a = nc.dram_tensor("w1a", [HIDDEN, 4 * HIDDEN], bf16, kind="ExternalInput")
    w1b = nc.dram_tensor("w1b", [HIDDEN, 4 * HIDDEN], bf16, kind="ExternalInput")
    w2s = nc.dram_tensor("w2s", [HIDDEN, 4 * HIDDEN], bf16, kind="ExternalInput")
    w3t = nc.dram_tensor("w3t", [HIDDEN, 2], bf16, kind="ExternalInput")
    b1t = nc.dram_tensor("b1t", [128, 4], f32, kind="ExternalInput")
    b2t = nc.dram_tensor("b2t", [128, 1], f32, kind="ExternalInput")
    out_t = nc.dram_tensor("out_t", [2, E_PAD], f32, kind="ExternalOutput")

    with tile.TileContext(nc) as tc:
        with (
            tc.tile_pool(name="const", bufs=1) as cp,
            tc.tile_pool(name="idx", bufs=3) as ip,
            tc.tile_pool(name="gath", bufs=3) as gp,
            tc.tile_pool(name="feat", bufs=3) as fp_,
            tc.tile_pool(name="act", bufs=3) as ap_,
            tc.tile_pool(name="outp", bufs=2) as op_,
            tc.tile_pool(name="ps_t", bufs=1, space="PSUM") as pst,
            tc.tile_pool(name="ps_h1", bufs=2, space="PSUM") as psh1,
            tc.tile_pool(name="ps_h2", bufs=2, space="PSUM") as psh2,
        ):
            ident = cp.tile([128, 128], bf16)
            make_identity(nc, ident[:])
            w1a_sb = cp.tile([128, 512], bf16)
            nc.sync.dma_start(out=w1a_sb[:], in_=w1a[:])
            w1b_sb = cp.tile([128, 512], bf16)
            nc.sync.dma_start(out=w1b_sb[:], in_=w1b[:])
            w2_sb = cp.tile([128, 512], bf16)
            nc.sync.dma_start(out=w2_sb[:], in_=w2s[:])
            w3_sb = cp.tile([128, 2], bf16)
            nc.sync.dma_start(out=w3_sb[:], in_=w3t[:])
            b1_sb = cp.tile([128, 4], f32)
            nc.sync.dma_start(out=b1_sb[:], in_=b1t[:])
            b2_sb = cp.tile([128, 1], f32)
            nc.sync.dma_start(out=b2_sb[:], in_=b2t[:])

            live = {}  # chunk -> (colT, rowT)

            def fetch(c):
                bkt = c // CHUNKS_PER_BUCKET
                ch, rh = bkt & 1, bkt >> 1
                csrc = emb16[SPLIT:TAB_PAD] if ch else emb16[0:SPLIT]
                rsrc = emb16[SPLIT:TAB_PAD] if rh else emb16[0:SPLIT]

                ic = ip.tile([128, G_CHUNK // 16], i16, tag="ic")
                nc.sync.dma_start(out=ic[:], in_=idx_col[c])
                ir = ip.tile([128, G_CHUNK // 16], i16, tag="ir")
                nc.sync.dma_start(out=ir[:], in_=idx_row[c])

                gc = gp.tile([128, NGRP, 128], bf16, tag="gc")
                nc.gpsimd.dma_gather(
                    gc[:], csrc, ic[:], G_CHUNK, G_CHUNK, HIDDEN,
                    transpose=False, queue_num=0,
                )
                gr = gp.tile([128, NGRP, 128], bf16, tag="gr")
                nc.gpsimd.dma_gather(
                    gr[:], rsrc, ir[:], G_CHUNK, G_CHUNK, HIDDEN,
                    transpose=False, queue_num=1,
                )

                colT = fp_.tile([128, G_CHUNK], bf16, tag="colT")
                rowT = fp_.tile([128, G_CHUNK], bf16, tag="rowT")
                for side, (g, dst) in enumerate(
                    ((gc, colT), (gr, rowT))
                ):
                    tpA = pst.tile([128, 512], bf16, tag=f"tpA{side}")
                    tpB = pst.tile([128, 384], bf16, tag=f"tpB{side}")
                    for t in range(NGRP):
                        tp = tpA[:, t * 128:(t + 1) * 128] if t < 4 else \
                            tpB[:, (t - 4) * 128:(t - 3) * 128]
                        nc.tensor.transpose(out=tp, in_=g[:, t], identity=ident[:])
                    # paired PSUM->SBUF copies, split across engines
                    if side == 0:
                        nc.vector.tensor_copy(out=dst[:, 0:512], in_=tpA[:])
                        nc.scalar.activation(out=dst[:, 512:896], in_=tpB[:],
                                             func=Ident)
                    else:
                        nc.scalar.activation(out=dst[:, 0:512], in_=tpA[:],
                                             func=Ident)
                        nc.vector.tensor_copy(out=dst[:, 512:896], in_=tpB[:])
                live[c] = (colT, rowT)

            def mlp(c):
                colT, rowT = live.pop(c)
                o_sb = op_.tile([2, G_CHUNK], f32, tag="o_sb")
                for blk in range(BLK_PER_CHUNK):
                    sl = slice(blk * BLOCK, (blk + 1) * BLOCK)
                    h1T = ap_.tile([128, 4 * BLOCK], bf16, tag="h1T")
                    for m in range(4):
                        h1p = psh1.tile([128, BLOCK], f32, tag="h1p")
                        nc.tensor.matmul(
                            out=h1p[:],
                            lhsT=w1a_sb[:, m * 128:(m + 1) * 128],
                            rhs=colT[:, sl],
                            start=True,
                            stop=False,
                        )
                        nc.tensor.matmul(
                            out=h1p[:],
                            lhsT=w1b_sb[:, m * 128:(m + 1) * 128],
                            rhs=rowT[:, sl],
                            start=False,
                            stop=True,
                        )
                        dst = h1T[:, m * BLOCK:(m + 1) * BLOCK]
                        if m % 2 == 0:
                            nc.scalar.activation(
                                out=dst, in_=h1p[:], func=Relu,
                                bias=b1_sb[:, m:m + 1],
                            )
                        else:
                            nc.vector.tensor_scalar(
                                out=dst, in0=h1p[:],
                                scalar1=b1_sb[:, m:m + 1], scalar2=0.0,
                                op0=ADD, op1=MAX,
                            )

                    h2p = psh2.tile([128, BLOCK], f32, tag="h2p")
                    for k in range(4):
                        nc.tensor.matmul(
                            out=h2p[:],
                            lhsT=w2_sb[:, k * 128:(k + 1) * 128],
                            rhs=h1T[:, k * BLOCK:(k + 1) * BLOCK],
                            start=(k == 0),
                            stop=(k == 3),
                        )
                    h2T = ap_.tile([128, BLOCK], bf16, tag="h2T")
                    nc.vector.tensor_scalar(
                        out=h2T[:], in0=h2p[:],
                        scalar1=b2_sb[:, 0:1], scalar2=0.0,
                        op0=ADD, op1=MAX,
                    )

                    op = psh2.tile([2, BLOCK], f32, tag="h2p")
                    nc.tensor.matmul(
                        out=op[:], lhsT=w3_sb[:], rhs=h2T[:],
                        start=True, stop=True,
                    )
                    nc.scalar.activation(out=o_sb[:, sl], in_=op[:], func=Ident)

                nc.sync.dma_start(
                    out=out_t[:, c * G_CHUNK:(c + 1) * G_CHUNK], in_=o_sb[:]
                )

            # software pipeline: fetch chunk c while the MLP consumes c-1
            for c in range(N_CHUNKS + 1):
                if c < N_CHUNKS:
                    fetch(c)
                if c >= 1:
                    mlp(c - 1)

    nc.compile()
    return nc


def _get_program():
    if "v4" not in _prog_cache:
        _prog_cache["v4"] = _build_program_v4()
    return _prog_cache["v4"]


def _gather_idx_layout(lidx):
    """[E_PAD] local indices -> [N_CHUNKS, 128, G_CHUNK//16] int16 in the
    dma_gather wrapped layout (element i of a chunk at [i%16, i//16],
    replicated across the 8 16-partition groups)."""
    a = lidx.astype(np.int16).reshape(N_CHUNKS, G_CHUNK // 16, 16)
    a = a.transpose(0, 2, 1)  # [N_CHUNKS, 16, G_CHUNK//16]
    return np.ascontiguousarray(np.tile(a, (1, 8, 1)))


def _marshal_core(col, row):
    """col/row: int32 [E_SHARD]. Returns (inputs, order, slots)."""
    bucket = (row >= SPLIT).astype(np.int64) * 2 + (col >= SPLIT)
    counts = np.bincount(bucket, minlength=N_BUCKETS)
    if counts.max() > CAP:
        raise RuntimeError(f"bucket overflow: {counts} > {CAP}")
    order = np.argsort(bucket, kind="stable")
    starts = np.concatenate([[0], np.cumsum(counts)[:-1]])
    within = np.arange(len(order)) - np.repeat(starts, counts)
    slots = np.repeat(np.arange(N_BUCKETS) * CAP, counts) + within

    lcol = np.zeros(E_PAD, np.int32)
    lrow = np.zeros(E_PAD, np.int32)
    bs = bucket[order]
    lcol[slots] = col[order] - SPLIT * (bs & 1)
    lrow[slots] = row[order] - SPLIT * (bs >> 1)
    return {
        "idx_col": _gather_idx_layout(lcol),
        "idx_row": _gather_idx_layout(lrow),
    }, order, slots


def kernel(emb, edge_index, W1, b1, W2, b2, W3, b3):
    if _REPO not in sys.path:
        sys.path.insert(0, _REPO)
    import ml_dtypes
    from concourse.bass_utils import run_bass_kernel_spmd

    bf = ml_dtypes.bfloat16
    emb = np.ascontiguousarray(np.asarray(emb, dtype=np.float32))
    emb16 = np.zeros((TAB_PAD, HIDDEN), bf)
    emb16[:N_NODES] = emb.astype(bf)
    ei = np.asarray(edge_index)
    col = ei[0].astype(np.int32)
    row = ei[1].astype(np.int32)
    W1 = np.asarray(W1, np.float32)
    w1a = np.ascontiguousarray(W1[:128].astype(bf))
    w1b = np.ascontiguousarray(W1[128:].astype(bf))
    W2 = np.asarray(W2, np.float32)
    # w2s[:, k*128:(k+1)*128] = W2[k*128:(k+1)*128, :]
    w2s = np.concatenate([W2[k * 128:(k + 1) * 128] for k in range(4)], axis=1)
    w2s = np.ascontiguousarray(w2s.astype(bf))
    w3t = np.ascontiguousarray(np.asarray(W3, np.float32).astype(bf))
    b1t = np.ascontiguousarray(np.asarray(b1, np.float32).reshape(4, 128).T)
    b2t = np.ascontiguousarray(np.asarray(b2, np.float32).reshape(128, 1))
    b3v = np.asarray(b3, np.float32).reshape(1, 2)

    in_maps = []
    unperm = []
    for i in range(N_CORES):
        m, order, slots = _marshal_core(
            col[i * E_SHARD:(i + 1) * E_SHARD],
            row[i * E_SHARD:(i + 1) * E_SHARD],
        )
        m.update(emb16=emb16, w1a=w1a, w1b=w1b, w2s=w2s, w3t=w3t,
                 b1t=b1t, b2t=b2t)
        in_maps.append(m)
        unperm.append((order, slots))

    nc = _get_program()
    try:
        res = run_bass_kernel_spmd(nc, in_maps, list(range(N_CORES)), **RUN_KWARGS)
    except Exception:
        import ctypes

        lib = ctypes.CDLL("/opt/axon/libaxon_pjrt.so")
        lib.axon_reset.restype = ctypes.c_int64
        lib.axon_reset()
        res = run_bass_kernel_spmd(nc, in_maps, list(range(N_CORES)), **RUN_KWARGS)
    global LAST_RESULTS
    LAST_RESULTS = res

    out = np.empty((N_EDGES, 2), np.float32)
    for i in range(N_CORES):
        ot = np.asarray(res.results[i]["out_t"], np.float32)  # [2, E_PAD]
        order, slots = unperm[i]
        shard = out[i * E_SHARD:(i + 1) * E_SHARD]
        shard[order] = ot.T[slots]
    out += b3v
    return out
